# revision 1
# baseline (speedup 1.0000x reference)
"""GCN 2-layer + mean-pool + log_softmax kernel for 8x TRN2 cores.

Strategy:
  - nodes sharded 8 ways (12544 padded nodes/core, 98 blocks of 128)
  - xw = x@W via PE (x^T supplied by host), y = xw*dinv (bf16 gather table)
  - AllGather y -> padded 256B-stride DRAM table per core
  - edge aggregation: dma_gather (bf16, 128B payload) by src + one-hot
    S-matmul scatter-add into PSUM per dst block; self-loop via identity
    matmul of local y; h = relu(dinv * psum)
  - pooling via one-hot batch matmul accumulated over all blocks + AllReduce
"""
import sys
for _p in ("/opt/trn_rl_repo", "/root/.axon_site/_ro/trn_rl_repo"):
    if _p not in sys.path:
        sys.path.append(_p)
import inspect
import numpy as np
import ml_dtypes

import concourse.bass as bass
import concourse.bacc as bacc
import concourse.mybir as mybir
import concourse.tile as tile

BF = ml_dtypes.bfloat16
P = 128
NCORES = 8
IN_CH = 256
HID = 32
OUT = 8
NG = 64
SENT = 1000.0  # one-hot sentinel (not in [0,128) / [0,64))


def _install_patched_gather():
    if hasattr(bass.BassGpSimd, "dma_gather_p"):
        return True
    try:
        src = inspect.getsource(bass.BassGpSimd.dma_gather)
        src = src.replace(
            "elem_size_bytes > 0 and elem_size_bytes % 256 == 0",
            "elem_size_bytes > 0 and elem_size_bytes % 32 == 0")
        src = "def dma_gather_p" + src[src.index("("):]
        ns = dict(bass.__dict__)
        exec(compile(src, "dma_gather_p", "exec"), ns)
        bass.BassGpSimd.dma_gather_p = ns["dma_gather_p"]
        return True
    except Exception:
        return False


class Plan:
    """Uniform (core-independent) edge schedule."""
    def __init__(self, Np, bpc, chunks_bg, call_plan, elem):
        self.Np = Np                  # padded node count
        self.bpc = bpc                # blocks per core
        self.npc = bpc * P            # nodes per core
        self.chunks_bg = chunks_bg    # [bpc][ngroups] chunk counts
        self.call_plan = call_plan    # per group: list of (q_start, nchunks)
        self.ngroups = len(call_plan)
        self.elem = elem              # gather elem_size (bf16 elements)
        self.gsize = -(-Np // self.ngroups)  # nodes per src group
        # derived
        self.nc_total = int(sum(sum(r) for r in chunks_bg))
        # chunk (b,g,j) -> q_g (index within group, block-major)
        self.prefix_g = np.zeros((bpc + 1, self.ngroups), np.int64)
        for b in range(bpc):
            for g in range(self.ngroups):
                self.prefix_g[b + 1, g] = self.prefix_g[b, g] + chunks_bg[b][g]
        # per group: cumulative idx-col offset of each call in the concat input
        self.call_col = {}
        col = 0
        for g in range(self.ngroups):
            lst = []
            for (q0, nch) in call_plan[g]:
                lst.append(col)
                col += nch * P // 16
            self.call_col[g] = lst
        self.idx_cols = col


def host_prep(x, edge_index, batch, ncores=NCORES, ngroups=4, elem=64,
              max_call_chunks=96):
    """Build the uniform schedule + per-core input arrays."""
    N = x.shape[0]
    src = np.asarray(edge_index[0], np.int64)
    dst = np.asarray(edge_index[1], np.int64)
    batch = np.asarray(batch, np.int64)

    npc = -(-N // (ncores * P)) * P        # nodes per core (128-mult)
    Np = npc * ncores
    bpc = npc // P
    gsize = -(-Np // ngroups)
    assert gsize <= 32768

    core = dst // npc
    blk = (dst % npc) // P
    # grp assigned after permutation (needs table rows)

    # ---- block->slot permutation balancing: sort each core's blocks by
    # total in-edge count so the per-slot max over cores is minimized ----
    cnt_cb = np.bincount(core * bpc + blk, minlength=ncores * bpc).reshape(ncores, bpc)
    perm = np.argsort(-cnt_cb, axis=1, kind="stable")      # slot s -> real block
    invperm = np.argsort(perm, axis=1)                     # real block -> slot
    slot = invperm[core, blk]
    # node -> table row (permuted layout)
    src_row = (src // npc) * npc + invperm[src // npc, (src % npc) // P] * P + src % P
    grp = src_row // gsize

    key = ((core * bpc + slot) * ngroups + grp)
    order = np.lexsort((src_row, key))      # sort by (core, slot, grp), then src row
    srcrow_s, dst_s = src_row[order], dst[order]
    key_s = key[order]

    nkeys = ncores * bpc * ngroups
    cnt = np.bincount(key_s, minlength=nkeys).reshape(ncores, bpc, ngroups)
    seg_end = np.cumsum(cnt.reshape(-1))
    seg_start = seg_end - cnt.reshape(-1)

    chunks_bg = np.maximum(-(-cnt.max(axis=0) // P), 0)  # [bpc, ngroups]
    # layer structure: every (b,g) keeps its chunks even if zero
    nc_total = int(chunks_bg.sum())

    # call plan per group: split group's chunk sequence into calls
    call_plan = []
    for g in range(ngroups):
        ncg = int(chunks_bg[:, g].sum())
        calls, q = [], 0
        while q < ncg:
            n = min(max_call_chunks, ncg - q)
            calls.append((q, n))
            q += n
        call_plan.append(calls)

    plan = Plan(Np, bpc, chunks_bg.tolist(), call_plan, elem)

    deg_full = np.bincount(dst, minlength=N).astype(np.float32) + 1.0

    # per-core padded arrays
    per_core = []
    for c in range(ncores):
        idx_flat = np.zeros((nc_total * P,), np.int16)        # pad idx -> 0
        dl_flat = np.full((nc_total * P,), SENT, np.float32)  # pad -> sentinel
        # fill segments: group-major position is NOT what we store;
        # dstloc columns are in (b-major, g, j) order == storage order here
        pos = 0
        gpos = np.zeros(ngroups, np.int64)  # per-group chunk counter (for idx)
        idx_groups = [np.zeros((int(chunks_bg[:, g].sum()) * P,), np.int16)
                      for g in range(ngroups)]
        for b in range(bpc):
            for g in range(ngroups):
                k = int(chunks_bg[b, g])
                if k == 0:
                    continue
                s0 = seg_start[(c * bpc + b) * ngroups + g]
                s1 = seg_end[(c * bpc + b) * ngroups + g]
                n = s1 - s0
                seg_idx = (srcrow_s[s0:s1] - g * gsize).astype(np.int16)
                seg_dl = (dst_s[s0:s1] % P).astype(np.float32)
                idx_groups[g][gpos[g] * P: gpos[g] * P + n] = seg_idx
                dl_flat[pos * P: pos * P + n] = seg_dl
                gpos[g] += k
                pos += k
        assert pos == nc_total

        # dstloc tile [P, NC]: column q = chunk q, partition = lane
        dstloc = dl_flat.reshape(nc_total, P).T.copy()

        # idx input: per group, per call, wrapped [16, nid/16] tiled to 128
        cols = []
        for g in range(ngroups):
            arr = idx_groups[g]
            for (q0, nch) in call_plan[g]:
                seg = arr[q0 * P:(q0 + nch) * P]
                nid = nch * P
                w = np.zeros((16, nid // 16), np.int16)
                ii = np.arange(nid)
                w[ii % 16, ii // 16] = seg
                cols.append(np.tile(w, (8, 1)))
        idx_in = np.concatenate(cols, axis=1) if cols else np.zeros((P, 1), np.int16)

        # per-core node data
        nbase = c * npc
        degc = np.ones((npc,), np.float32)
        hi = min(nbase + npc, N)
        if hi > nbase:
            degc[:hi - nbase] = deg_full[nbase:hi]
        bl = np.full((npc,), SENT, np.float32)
        if hi > nbase:
            bl[:hi - nbase] = batch[nbase:hi].astype(np.float32)
        # permute blocks into slot order
        degc = degc.reshape(bpc, P)[perm[c]].reshape(npc)
        bl = bl.reshape(bpc, P)[perm[c]].reshape(npc)
        deg_t = degc.reshape(bpc, P).T.copy()          # [P, bpc]
        bl_t = bl.reshape(bpc, P).T.copy()          # [P, bpc]

        per_core.append(dict(idx=idx_in, dstloc=dstloc, deg=deg_t, batchloc=bl_t))

    cnts = np.bincount(batch, minlength=NG).astype(np.float32).reshape(NG, 1)
    xT = np.zeros((IN_CH, Np), BF)
    xT[:, :N] = np.asarray(x, np.float32).T.astype(BF)
    # permute each core's columns into slot order
    colperm = np.empty((Np,), np.int64)
    for c in range(ncores):
        base = c * npc
        colperm[base:base + npc] = base + (perm[c][:, None] * P +
                                           np.arange(P)[None, :]).reshape(-1)
    xT = xT[:, colperm]
    return plan, per_core, cnts, xT


def build(plan: Plan, with_b1, with_b2, debug=False, reps=1):
    use_patch = plan.elem * 2 % 256 != 0
    if use_patch:
        assert _install_patched_gather()
    nc = bacc.Bacc("TRN2", target_bir_lowering=False, debug=False,
                   num_swdge_queues=1, dynamic_dma_scratch_size=65536)
    f32, bf16, i16, i32 = (mybir.dt.float32, mybir.dt.bfloat16,
                           mybir.dt.int16, mybir.dt.int32)
    fp8 = mybir.dt.float8e4
    A = mybir.AluOpType
    AF = mybir.ActivationFunctionType
    npc, bpc, NC, EL = plan.npc, plan.bpc, plan.nc_total, plan.elem
    Np = plan.Np
    TABW = 256  # fp8 elems per table row (256B stride)

    xT = nc.dram_tensor("xT", [IN_CH, npc], bf16, kind="ExternalInput")
    W1 = nc.dram_tensor("W1", [IN_CH, HID], f32, kind="ExternalInput")
    W2 = nc.dram_tensor("W2", [HID, OUT], f32, kind="ExternalInput")
    b1 = nc.dram_tensor("b1", [1, HID], f32, kind="ExternalInput")
    b2 = nc.dram_tensor("b2", [1, OUT], f32, kind="ExternalInput")
    deg = nc.dram_tensor("deg", [P, bpc], f32, kind="ExternalInput")
    dstloc = nc.dram_tensor("dstloc", [P, NC], f32, kind="ExternalInput")
    batchloc = nc.dram_tensor("batchloc", [P, bpc], f32, kind="ExternalInput")
    cnts = nc.dram_tensor("cnts", [NG, 1], f32, kind="ExternalInput")
    idx = nc.dram_tensor("idx", [P, plan.idx_cols], i16, kind="ExternalInput")
    out = nc.dram_tensor("out", [NG, OUT], f32, kind="ExternalOutput")
    if debug:
        dbgo_y1 = nc.dram_tensor("dbg_y1", [P, bpc * HID], bf16, kind="ExternalOutput")
        dbgo_h1 = nc.dram_tensor("dbg_h1", [P, bpc * HID], bf16, kind="ExternalOutput")
        dbgo_y2 = nc.dram_tensor("dbg_y2", [P, bpc * OUT], bf16, kind="ExternalOutput")
        dbgo_h2 = nc.dram_tensor("dbg_h2", [P, bpc * OUT], bf16, kind="ExternalOutput")
        dbgo_S = nc.dram_tensor("dbg_S", [P, P], bf16, kind="ExternalOutput")
        dbgo_rhs = nc.dram_tensor("dbg_rhs", [P, HID], bf16, kind="ExternalOutput")
        dbgo_tab = nc.dram_tensor("dbg_tab", [P, TABW], bf16, kind="ExternalOutput")

    with tile.TileContext(nc) as tc:
        with tc.tile_pool(name="const", bufs=1) as cpool, \
             tc.tile_pool(name="persist", bufs=1) as pers, \
             tc.tile_pool(name="gth", bufs=2) as gpool, \
             tc.tile_pool(name="spool", bufs=16) as spool, \
             tc.tile_pool(name="psA", bufs=2, space="PSUM") as psA, \
             tc.tile_pool(name="psB", bufs=4, space="PSUM") as psB, \
             tc.tile_pool(name="psE", bufs=1, space="PSUM") as psE, \
             tc.tile_pool(name="dram", bufs=1, space="DRAM") as dram:

            # ---- constants ----
            iota_i = cpool.tile([P, P], i32)
            nc.gpsimd.iota(iota_i[:], pattern=[[1, P]], base=0, channel_multiplier=0)
            iota_f = cpool.tile([P, P], f32)
            nc.vector.tensor_copy(out=iota_f[:], in_=iota_i[:])
            iota_bf = cpool.tile([P, P], bf16)
            nc.vector.tensor_copy(out=iota_bf[:], in_=iota_f[:])
            giota_bf = cpool.tile([P, NG], bf16)
            nc.vector.tensor_copy(out=giota_bf[:], in_=iota_f[:, :NG])
            ident_f = cpool.tile([P, P], f32)
            nc.gpsimd.memset(ident_f[:], 0.0)
            nc.gpsimd.affine_select(
                out=ident_f[:], in_=ident_f[:], compare_op=A.not_equal,
                fill=1.0, base=0, pattern=[[-1, P]], channel_multiplier=1)
            ident_bf = cpool.tile([P, P], bf16)
            nc.vector.tensor_copy(out=ident_bf[:], in_=ident_f[:])
            ident_f8 = cpool.tile([P, P], fp8)
            nc.vector.tensor_copy(out=ident_f8[:], in_=ident_f[:])

            w1f = cpool.tile([P, 2 * HID], f32)
            nc.sync.dma_start(out=w1f[:, 0:HID], in_=W1[0:P, :])
            nc.sync.dma_start(out=w1f[:, HID:2 * HID], in_=W1[P:2 * P, :])
            w1t = cpool.tile([P, 2 * HID], bf16)
            nc.vector.tensor_copy(out=w1t[:], in_=w1f[:])
            w2f = cpool.tile([HID, OUT], f32)
            nc.sync.dma_start(out=w2f[:], in_=W2[:, :])
            w2t = cpool.tile([HID, OUT], bf16)
            nc.vector.tensor_copy(out=w2t[:], in_=w2f[:])
            b1t = cpool.tile([1, HID], f32)
            nc.sync.dma_start(out=b1t[:], in_=b1[:, :])
            b2t = cpool.tile([1, OUT], f32)
            nc.sync.dma_start(out=b2t[:], in_=b2[:, :])

            deg_t = cpool.tile([P, bpc], f32)
            nc.sync.dma_start(out=deg_t[:], in_=deg[:, :])
            rdeg = cpool.tile([P, bpc], f32)
            nc.vector.reciprocal(out=rdeg[:], in_=deg_t[:])
            dinv = cpool.tile([P, bpc], f32)
            nc.scalar.activation(out=dinv[:], in_=rdeg[:], func=AF.Sqrt)

            dl_t = pers.tile([P, NC], f32)
            nc.sync.dma_start(out=dl_t[:], in_=dstloc[:, :])
            dlneg = pers.tile([P, NC], f32)
            nc.vector.tensor_scalar(out=dlneg[:], in0=dl_t[:], scalar1=-1.0,
                                    scalar2=None, op0=A.mult)
            bl_t = cpool.tile([P, bpc], f32)
            nc.sync.dma_start(out=bl_t[:], in_=batchloc[:, :])
            idx_t = pers.tile([P, plan.idx_cols], i16)
            nc.sync.dma_start(out=idx_t[:], in_=idx[:, :])

            # bias broadcast tiles (built only if needed)
            if with_b1:
                b1b_ps = psA.tile([P, HID], f32, tag="pst")
                ones_col = cpool.tile([1, P], f32)
                nc.gpsimd.memset(ones_col[:], 1.0)
                nc.tensor.matmul(out=b1b_ps[:], lhsT=ones_col[:], rhs=b1t[:],
                                 start=True, stop=True)
                b1b = cpool.tile([P, HID], f32)
                nc.vector.tensor_copy(out=b1b[:], in_=b1b_ps[:])
            if with_b2:
                b2b_ps = psA.tile([P, OUT], f32, tag="pst")
                ones_col2 = cpool.tile([1, P], f32)
                nc.gpsimd.memset(ones_col2[:], 1.0)
                nc.tensor.matmul(out=b2b_ps[:], lhsT=ones_col2[:], rhs=b2t[:],
                                 start=True, stop=True)
                b2b = cpool.tile([P, OUT], f32)
                nc.vector.tensor_copy(out=b2b[:], in_=b2b_ps[:])

            # ---- persistent activations ----
            y1_sh = pers.tile([P, bpc * HID], fp8)
            h1_sh = pers.tile([P, bpc * HID], bf16)
            y2_sh = pers.tile([P, bpc * OUT], fp8)
            h2_sh = pers.tile([P, bpc * OUT], bf16)

            # ================= repeated pipeline (timing) ================
            for _rep in range(reps):
              # ---- DRAM internals (per rep: Shared tiles are single-writer) ----
              y1cc = dram.tile([P, bpc * HID], fp8, name=f"y1cc{_rep}")
              y1full = dram.tile([NCORES * P, bpc * HID], fp8, addr_space="Shared", name=f"y1full{_rep}")
              y1tab = dram.tile([Np, TABW], fp8, name=f"y1tab{_rep}")
              y2cc = dram.tile([P, bpc * OUT], fp8, name=f"y2cc{_rep}")
              y2full = dram.tile([NCORES * P, bpc * OUT], fp8, addr_space="Shared", name=f"y2full{_rep}")
              y2tab = dram.tile([Np, TABW], fp8, name=f"y2tab{_rep}")
              pcc_i = dram.tile([NG, OUT], f32, name=f"pcc_i{_rep}")
              pcc_o = dram.tile([NG, OUT], f32, addr_space="Shared", name=f"pcc_o{_rep}")
              # ---- stage A: y1 = (x @ W1) * dinv ----
              SLAB = 8  # blocks per xT slab
              sbA_ctx = tc.tile_pool(name="sbA", bufs=2)
              sbA = sbA_ctx.__enter__()
              for s0 in range(0, bpc, SLAB):
                  sb = min(SLAB, bpc - s0)
                  xt0 = sbA.tile([P, SLAB * P], bf16, tag="xt0")
                  xt1 = sbA.tile([P, SLAB * P], bf16, tag="xt1")
                  nc.sync.dma_start(out=xt0[:, :sb * P], in_=xT[0:P, s0 * P:(s0 + sb) * P])
                  nc.sync.dma_start(out=xt1[:, :sb * P], in_=xT[P:2 * P, s0 * P:(s0 + sb) * P])
                  for j in range(sb):
                      b = s0 + j
                      ps = psA.tile([P, HID], f32, name="psA_t", tag="pst")
                      nc.tensor.matmul(out=ps[:], lhsT=xt0[:, j * P:(j + 1) * P],
                                       rhs=w1t[:, 0:HID], start=True, stop=False)
                      nc.tensor.matmul(out=ps[:], lhsT=xt1[:, j * P:(j + 1) * P],
                                       rhs=w1t[:, HID:2 * HID], start=False, stop=True)
                      nc.vector.tensor_scalar(
                          out=y1_sh[:, b * HID:(b + 1) * HID], in0=ps[:],
                          scalar1=dinv[:, b:b + 1], scalar2=None, op0=A.mult)

              sbA_ctx.__exit__(None, None, None)
              nc.sync.dma_start(out=y1cc[:], in_=y1_sh[:])
              import os as _os
              if int(_os.environ.get("GNN_SKIP_CC", "0")):
                  nc.gpsimd.dma_start(out=y1full[0:P, :], in_=y1cc[:])
              else:
                  nc.gpsimd.collective_compute(
                      "AllGather", A.bypass, ins=[y1cc[:]], outs=[y1full[:]],
                      replica_groups=[list(range(NCORES))])
              for cexp in range(NCORES):
                  nc.gpsimd.dma_start(
                      out=y1tab[cexp * npc:(cexp + 1) * npc, 0:HID].rearrange(
                          "(b p) ch -> b p ch", p=P),
                      in_=y1full[cexp * P:(cexp + 1) * P, :].rearrange(
                          "p (b ch) -> b p ch", ch=HID))

              # ---- aggregation layers ----
              dbg_state = {"todo": debug, "S": dbgo_S, "rhs": dbgo_rhs} if debug else {}
              def agg_layer(ytab, ysh, hsh, C, with_b, bb):
                  # issue gather calls lazily; track per-group state
                  gstate = [dict(call=-1, tile=None) for _ in range(plan.ngroups)]

                  def ensure_call(g, q):
                      cidx = 0
                      for i, (q0, nch) in enumerate(plan.call_plan[g]):
                          if q0 <= q < q0 + nch:
                              cidx = i
                              break
                      st = gstate[g]
                      if st["call"] == cidx:
                          return st["tile"], plan.call_plan[g][cidx][0]
                      (q0, nch) = plan.call_plan[g][cidx]
                      gt = gpool.tile([P, 96 * EL], fp8, tag=f"g{g}", name=f"gt{g}")
                      col = plan.call_col[g][cidx]
                      nid = nch * P
                      gather = nc.gpsimd.dma_gather_p if use_patch else nc.gpsimd.dma_gather
                      gather(
                          out_ap=gt[:, :nch * EL].rearrange("p (k c) -> p k c", c=EL),
                          in_ap=ytab[g * plan.gsize: min((g + 1) * plan.gsize, plan.Np), 0:EL],
                          idxs_ap=idx_t[:, col:col + nid // 16],
                          num_idxs=nid, num_idxs_reg=nid,
                          elem_size=EL, elem_step=TABW, single_packet=False)
                      st["call"] = cidx
                      st["tile"] = gt
                      return gt, q0

                  import os
                  skip_chunks = bool(int(os.environ.get("GNN_SKIP_CHUNKS", "0")))
                  fixed_s = bool(int(os.environ.get("GNN_FIXED_S", "0")))
                  skip_gather = bool(int(os.environ.get("GNN_SKIP_GATHER", "0")))
                  qcol = 0  # dstloc column counter (b-major)
                  for b in range(bpc):
                      ps = psB.tile([P, C], f32, name="psB_t")
                      nchunks = 0 if skip_chunks else sum(plan.chunks_bg[b])
                      nc.tensor.matmul(out=ps[:], lhsT=ident_bf[:],
                                       rhs=ysh[:, b * C:(b + 1) * C],
                                       start=True, stop=(nchunks == 0))
                      done = 0
                      for g in range(plan.ngroups):
                          for j in range(0 if skip_chunks else plan.chunks_bg[b][g]):
                              q = int(plan.prefix_g[b, g]) + j
                              if skip_gather:
                                  gt, q0 = None, q
                              else:
                                  gt, q0 = ensure_call(g, q)
                              slot = q - q0
                              if fixed_s:
                                  S = ident_bf
                              elif qcol % 6 == 2:
                                  S = spool.tile([P, P], bf16, name="Sg_t")
                                  nc.gpsimd.tensor_scalar(
                                      out=S[:], in0=iota_bf[:],
                                      scalar1=dl_t[:, qcol:qcol + 1], scalar2=None,
                                      op0=A.is_equal)
                              elif qcol % 6 == 5:
                                  ta = spool.tile([P, P], f32, name="ta_t")
                                  nc.scalar.activation(
                                      out=ta[:], in_=iota_f[:], func=AF.Square,
                                      bias=dlneg[:, qcol:qcol + 1], scale=1.0)
                                  S = spool.tile([P, P], bf16, name="Sa_t")
                                  nc.scalar.activation(
                                      out=S[:], in_=ta[:], func=AF.Relu,
                                      bias=1.0, scale=-1.0)
                              else:
                                  S = spool.tile([P, P], bf16, name="S_t")
                                  nc.vector.tensor_scalar(
                                      out=S[:], in0=iota_bf[:],
                                      scalar1=dl_t[:, qcol:qcol + 1], scalar2=None,
                                      op0=A.is_equal)
                              done += 1
                              if debug and dbg_state.get("todo"):
                                  nc.sync.dma_start(out=dbg_state["S"][:, :], in_=S[:])
                                  nc.sync.dma_start(out=dbg_state["rhs"][:, :],
                                                    in_=gt[:, slot * EL: slot * EL + C])
                                  dbg_state["todo"] = False
                              nc.tensor.matmul(
                                  out=ps[:], lhsT=S[:],
                                  rhs=(ysh[:, b * C:(b + 1) * C] if gt is None else
                                       gt[:, slot * EL: slot * EL + C]),
                                  start=False, stop=(done == nchunks))
                              qcol += 1
                      if with_b:
                          t1 = spool.tile([P, C], f32, name="t1_t")
                          nc.vector.tensor_scalar(
                              out=t1[:], in0=ps[:], scalar1=dinv[:, b:b + 1],
                              scalar2=None, op0=A.mult)
                          t2 = spool.tile([P, C], f32, name="t2_t")
                          nc.vector.tensor_tensor(out=t2[:], in0=t1[:],
                                                  in1=bb[:, :C], op=A.add)
                          nc.scalar.activation(out=hsh[:, b * C:(b + 1) * C],
                                               in_=t2[:], func=AF.Relu)
                      else:
                          nc.vector.tensor_scalar(
                              out=hsh[:, b * C:(b + 1) * C], in0=ps[:],
                              scalar1=dinv[:, b:b + 1], scalar2=0.0,
                              op0=A.mult, op1=A.max)

              agg_layer(y1tab, y1_sh, h1_sh, HID, with_b1, b1b if with_b1 else None)

              # ---- stage C: per-block transpose + xw2 ----
              for b in range(bpc):
                  tp = psA.tile([HID, P], bf16, name="psT_t", tag="pst")
                  nc.tensor.transpose(out=tp[:], in_=h1_sh[:, b * HID:(b + 1) * HID],
                                      identity=ident_bf[:])
                  h1Tb = spool.tile([HID, P], bf16, name="h1Tb_t")
                  nc.vector.tensor_copy(out=h1Tb[:], in_=tp[:])
                  ps2 = psA.tile([P, OUT], f32, name="ps2_t", tag="pst")
                  nc.tensor.matmul(out=ps2[:], lhsT=h1Tb[:],
                                   rhs=w2t[:], start=True, stop=True)
                  nc.vector.tensor_scalar(
                      out=y2_sh[:, b * OUT:(b + 1) * OUT], in0=ps2[:],
                      scalar1=dinv[:, b:b + 1], scalar2=None, op0=A.mult)

              nc.sync.dma_start(out=y2cc[:], in_=y2_sh[:])
              if int(_os.environ.get("GNN_SKIP_CC", "0")):
                  nc.gpsimd.dma_start(out=y2full[0:P, :], in_=y2cc[:])
              else:
                  nc.gpsimd.collective_compute(
                      "AllGather", A.bypass, ins=[y2cc[:]], outs=[y2full[:]],
                      replica_groups=[list(range(NCORES))])
              for cexp in range(NCORES):
                  nc.gpsimd.dma_start(
                      out=y2tab[cexp * npc:(cexp + 1) * npc, 0:OUT].rearrange(
                          "(b p) ch -> b p ch", p=P),
                      in_=y2full[cexp * P:(cexp + 1) * P, :].rearrange(
                          "p (b ch) -> b p ch", ch=OUT))

              agg_layer(y2tab, y2_sh, h2_sh, OUT, with_b2, b2b if with_b2 else None)

              if debug:
                  tb = cpool.tile([P, TABW], bf16, name="tbdump")
                  nc.sync.dma_start(out=tb[:], in_=y1tab[0:P, :])
                  nc.sync.dma_start(out=dbgo_tab[:, :], in_=tb[:])
                  nc.sync.dma_start(out=dbgo_y1[:, :], in_=y1_sh[:])

                  nc.sync.dma_start(out=dbgo_h1[:, :], in_=h1_sh[:])
                  nc.sync.dma_start(out=dbgo_y2[:, :], in_=y2_sh[:])
                  nc.sync.dma_start(out=dbgo_h2[:, :], in_=h2_sh[:])
              # ---- stage E: pooling + log_softmax ----
              pp = psE.tile([NG, OUT], f32)
              for b in range(bpc):
                  Sb = spool.tile([P, NG], bf16, name="Sb_t")
                  nc.vector.tensor_scalar(
                      out=Sb[:], in0=giota_bf[:], scalar1=bl_t[:, b:b + 1],
                      scalar2=None, op0=A.is_equal)
                  nc.tensor.matmul(out=pp[:], lhsT=Sb[:], rhs=h2_sh[:, b * OUT:(b + 1) * OUT],
                                   start=(b == 0), stop=(b == bpc - 1))
              sums = cpool.tile([NG, OUT], f32)
              nc.vector.tensor_copy(out=sums[:], in_=pp[:])
              nc.sync.dma_start(out=pcc_i[:], in_=sums[:])
              if int(_os.environ.get("GNN_SKIP_CC", "0")):
                  nc.gpsimd.dma_start(out=pcc_o[:], in_=pcc_i[:])
              else:
                  nc.gpsimd.collective_compute(
                      "AllReduce", A.add, ins=[pcc_i[:]], outs=[pcc_o[:]],
                      replica_groups=[list(range(NCORES))])
              gsum = cpool.tile([NG, OUT], f32)
              nc.sync.dma_start(out=gsum[:], in_=pcc_o[:])

              cnt_t = cpool.tile([NG, 1], f32)
              nc.sync.dma_start(out=cnt_t[:], in_=cnts[:, :])
              cmax = cpool.tile([NG, 1], f32)
              nc.vector.tensor_scalar(out=cmax[:], in0=cnt_t[:], scalar1=1.0,
                                      scalar2=None, op0=A.max)
              cinv = cpool.tile([NG, 1], f32)
              nc.vector.reciprocal(out=cinv[:], in_=cmax[:])
              pooled = cpool.tile([NG, OUT], f32)
              nc.vector.tensor_scalar(out=pooled[:], in0=gsum[:], scalar1=cinv[:, :1],
                                      scalar2=None, op0=A.mult)
              m = cpool.tile([NG, 1], f32)
              nc.vector.reduce_max(out=m[:], in_=pooled[:], axis=mybir.AxisListType.X)
              z = cpool.tile([NG, OUT], f32)
              nc.vector.tensor_scalar(out=z[:], in0=pooled[:], scalar1=m[:, :1],
                                      scalar2=None, op0=A.subtract)
              e = cpool.tile([NG, OUT], f32)
              nc.scalar.activation(out=e[:], in_=z[:], func=AF.Exp)
              ssum = cpool.tile([NG, 1], f32)
              nc.vector.reduce_sum(out=ssum[:], in_=e[:], axis=mybir.AxisListType.X)
              ls = cpool.tile([NG, 1], f32)
              nc.scalar.activation(out=ls[:], in_=ssum[:], func=AF.Ln)
              o = cpool.tile([NG, OUT], f32)
              nc.vector.tensor_scalar(out=o[:], in0=z[:], scalar1=ls[:, :1],
                                      scalar2=None, op0=A.subtract)
              nc.sync.dma_start(out=out[:, :], in_=o[:])

    nc.compile()
    return nc


def run_gnn(x, edge_index, batch, W1, b1, W2, b2, ngroups=4, elem=64,
            runner_cls=None, time_iters=0, debug=False, reps=1):
    from concourse.bass_utils import run_bass_kernel_spmd
    plan, per_core, cnts, xT = host_prep(x, edge_index, batch, ngroups=ngroups, elem=elem)
    with_b1 = bool(np.any(np.asarray(b1)))
    with_b2 = bool(np.any(np.asarray(b2)))
    nc = build(plan, with_b1, with_b2, debug=debug, reps=reps)
    in_maps = []
    for c in range(NCORES):
        m = dict(per_core[c])
        m["xT"] = np.ascontiguousarray(xT[:, c * plan.npc:(c + 1) * plan.npc])
        m["W1"] = np.asarray(W1, np.float32)
        m["W2"] = np.asarray(W2, np.float32)
        m["b1"] = np.asarray(b1, np.float32).reshape(1, HID)
        m["b2"] = np.asarray(b2, np.float32).reshape(1, OUT)
        m["cnts"] = cnts
        in_maps.append(m)
    if runner_cls is not None:
        r = runner_cls(nc, NCORES)
        args = r.put_inputs(in_maps)
        tinfo = r.time_execute(args, iters=time_iters) if time_iters else None
        outs = r.run(args)
        res = r.results(outs)
        return res[0]["out"], tinfo
    res = run_bass_kernel_spmd(nc, in_maps, core_ids=list(range(NCORES)))
    if debug:
        return res.results, None
    return res.results[0]["out"], None



def kernel(x, edge_index, batch, W1, b1, W2, b2):
    """Full-input 2-layer GCN + mean-pool + log_softmax on 8 trn2 NeuronCores."""
    import os
    for _v in ("GNN_SKIP_CHUNKS", "GNN_FIXED_S", "GNN_SKIP_GATHER", "GNN_SKIP_CC"):
        os.environ.pop(_v, None)
    out, _ = run_gnn(np.asarray(x), np.asarray(edge_index), np.asarray(batch),
                     np.asarray(W1), np.asarray(b1), np.asarray(W2), np.asarray(b2))
    return np.asarray(out, dtype=np.float32)



# revision 3
# speedup vs baseline: 1.6897x; 1.6897x over previous
"""GCN 2-layer + mean-pool + log_softmax kernel for 8x TRN2 cores.

Strategy (v1: fixed-S scatter structure):
  - nodes sharded 8 ways (12544 padded nodes/core, 98 blocks of 128)
  - xw = x@W via PE (x^T supplied by host), y = xw*dinv (fp8 table)
  - AllGather y -> per-core DRAM table with 256B row stride, grouped in 4
    windows of <=32768 rows (int16 gather index limit), each with a spare
    zero-row region for padding indices
  - edge aggregation: every dst lane owns K=4 fixed slots per source group;
    the scatter one-hot matrices are 4 CONSTANTS (no per-chunk vector work).
    Overflow edges go to dynamic spill chunks with on-the-fly one-hot.
  - dma_gather (fp8, 32B payload) by src; S-matmul scatter-add into PSUM per
    dst block; self-loop via identity matmul; h = relu(dinv * psum)
  - layer-1 block finalize immediately feeds transpose + x@W2 (stage C
    interleaved per block)
  - pooling via one-hot batch matmul accumulated over all blocks; partial
    per-core sums are returned; host does the cross-core sum + log_softmax.
"""
import sys
for _p in ("/opt/trn_rl_repo", "/root/.axon_site/_ro/trn_rl_repo"):
    if _p not in sys.path:
        sys.path.append(_p)
import inspect
import numpy as np
import ml_dtypes

import concourse.bass as bass
import concourse.bacc as bacc
import concourse.mybir as mybir
import concourse.tile as tile

BF = ml_dtypes.bfloat16
F8 = ml_dtypes.float8_e4m3
P = 128
NCORES = 8
IN_CH = 256
HID = 32
OUT = 8
NG = 64
NGROUPS = 4
KSLOT = 4              # fixed slots per (lane, group)
LPC = P // KSLOT       # lanes per regular chunk
ZROWS = 128            # spare zero rows per table window
SENT = 1000.0          # one-hot sentinel (not in [0,128) / [0,64))


def _install_patched_gather():
    if hasattr(bass.BassGpSimd, "dma_gather_p"):
        return True
    src = inspect.getsource(bass.BassGpSimd.dma_gather)
    src = src.replace(
        "elem_size_bytes > 0 and elem_size_bytes % 256 == 0",
        "elem_size_bytes > 0 and elem_size_bytes % 32 == 0")
    src = "def dma_gather_p" + src[src.index("("):]
    ns = dict(bass.__dict__)
    exec(compile(src, "dma_gather_p", "exec"), ns)
    bass.BassGpSimd.dma_gather_p = ns["dma_gather_p"]
    return True


class Plan:
    """Uniform (core-independent) chunk schedule."""
    def __init__(self, Np, bpc, nch_bg, call_plan, nsp_col_of, maxch):
        self.Np = Np
        self.bpc = bpc
        self.npc = bpc * P
        self.nch_bg = nch_bg            # [bpc][NGROUPS] chunks per (block, group)
        self.call_plan = call_plan      # per group: list of (q_start, nchunks)
        self.nsp_col_of = nsp_col_of    # dict (b, g) -> first spill dstloc col
        self.nsp_cols = max(nsp_col_of.values(), default=-1) + 1 if nsp_col_of else 0
        self.maxch = maxch
        self.gsize = Np // NGROUPS
        self.win = self.gsize + ZROWS   # table window stride (rows)
        self.prefix_g = np.zeros((bpc + 1, NGROUPS), np.int64)
        for b in range(bpc):
            self.prefix_g[b + 1] = self.prefix_g[b] + nch_bg[b]
        # per group: idx-col offset of each call in the concat idx input
        self.call_col = {}
        col = 0
        for g in range(NGROUPS):
            lst = []
            for (q0, nch) in call_plan[g]:
                lst.append(col)
                col += nch * P // 16
            self.call_col[g] = lst
        self.idx_cols = col


def host_prep(x, edge_index, batch, maxch=192):
    N = x.shape[0]
    src = np.asarray(edge_index[0], np.int64)
    dst = np.asarray(edge_index[1], np.int64)
    batch = np.asarray(batch, np.int64)

    npc = -(-N // (NCORES * P)) * P
    Np = npc * NCORES
    bpc = npc // P
    assert Np % NGROUPS == 0
    gsize = Np // NGROUPS
    assert gsize + ZROWS <= 32768

    core = dst // npc
    blk = (dst % npc) // P
    lane = dst % P

    # block -> slot permutation balancing total in-edge count across cores
    cnt_cb = np.bincount(core * bpc + blk, minlength=NCORES * bpc).reshape(NCORES, bpc)
    perm = np.argsort(-cnt_cb, axis=1, kind="stable")      # slot s -> real block
    invperm = np.argsort(perm, axis=1)                     # real block -> slot
    slot = invperm[core, blk]
    src_row = (src // npc) * npc + invperm[src // npc, (src % npc) // P] * P + src % P
    grp = src_row // gsize
    rel = (src_row - grp * gsize).astype(np.int64)

    # sort edges by (core, slot, group, lane)
    key = ((core * bpc + slot) * NGROUPS + grp) * P + lane
    order = np.argsort(key, kind="stable")
    key_s = key[order]
    rel_s = rel[order]
    nkeys = NCORES * bpc * NGROUPS * P
    cnt4 = np.bincount(key_s, minlength=nkeys)
    seg_start = np.concatenate([[0], np.cumsum(cnt4)])[:-1]
    ordinal = np.arange(len(key_s)) - seg_start[key_s]

    cnt4r = cnt4.reshape(NCORES, bpc, NGROUPS, P)
    spill = np.maximum(cnt4r - KSLOT, 0)
    spill_bg_c = spill.sum(axis=3)                        # [NCORES, bpc, NGROUPS]
    nspill_bg = -(-spill_bg_c.max(axis=0) // P)           # [bpc, NGROUPS]
    nch_bg = (KSLOT + nspill_bg).astype(np.int64)

    # spill dstloc columns ordered (b-major, then g, then chunk j)
    nsp_col_of = {}
    col = 0
    for b in range(bpc):
        for g in range(NGROUPS):
            if nspill_bg[b, g]:
                nsp_col_of[(b, g)] = col
                col += int(nspill_bg[b, g])
    nsp_cols = col

    # call plan per group
    chunks_g = (KSLOT + nspill_bg).sum(axis=0)            # [NGROUPS]
    call_plan = []
    for g in range(NGROUPS):
        ncg = int(chunks_g[g])
        calls, q = [], 0
        while q < ncg:
            n = min(maxch, ncg - q)
            calls.append((q, n))
            q += n
        call_plan.append(calls)

    plan = Plan(Np, bpc, nch_bg.tolist(), call_plan, nsp_col_of, maxch)
    prefix_g = plan.prefix_g

    deg_full = np.bincount(dst, minlength=N).astype(np.float32) + 1.0

    # decompose sorted edge keys once
    core_s = key_s // (bpc * NGROUPS * P)
    rem = key_s % (bpc * NGROUPS * P)
    slot_s = rem // (NGROUPS * P)
    g_s = (rem // P) % NGROUPS
    lane_s = rem % P

    per_core = []
    for c in range(NCORES):
        m = core_s == c
        sl, gg, ln, o, rr = slot_s[m], g_s[m], lane_s[m], ordinal[m], rel_s[m]

        idx_groups = [np.full((int(chunks_g[g]) * P,), gsize, np.int16)
                      for g in range(NGROUPS)]
        dl = np.full((max(nsp_cols, 1), P), SENT, np.float32)

        # regular slots
        regm = o < KSLOT
        q = prefix_g[sl[regm], gg[regm]] + ln[regm] // LPC
        pos = q * P + (ln[regm] % LPC) * KSLOT + o[regm]
        for g in range(NGROUPS):
            gm = gg[regm] == g
            idx_groups[g][pos[gm]] = rr[regm][gm].astype(np.int16)

        # spill slots: ordinal within (slot, group) among spill edges
        spm = ~regm
        if spm.any():
            key2 = sl[spm] * NGROUPS + gg[spm]
            cnt2 = np.bincount(key2, minlength=bpc * NGROUPS)
            st2 = np.concatenate([[0], np.cumsum(cnt2)])[:-1]
            so = np.arange(len(key2)) - st2[key2]
            qsp = prefix_g[sl[spm], gg[spm]] + KSLOT + so // P
            pos = qsp * P + so % P
            for g in range(NGROUPS):
                gm = gg[spm] == g
                idx_groups[g][pos[gm]] = rr[spm][gm].astype(np.int16)
            # dstloc
            colbase = np.array([nsp_col_of.get((b, g), 0)
                                for b in range(bpc) for g in range(NGROUPS)],
                               np.int64).reshape(bpc, NGROUPS)
            cols = colbase[sl[spm], gg[spm]] + so // P
            dl[cols, so % P] = ln[spm].astype(np.float32)

        dstloc = dl.T.copy()  # [P, nsp_cols]

        # idx input: per group, per call, wrapped [16, nid/16] tiled to 128
        cols_list = []
        for g in range(NGROUPS):
            arr = idx_groups[g]
            for (q0, nch) in call_plan[g]:
                seg = arr[q0 * P:(q0 + nch) * P]
                nid = nch * P
                w = np.zeros((16, nid // 16), np.int16)
                ii = np.arange(nid)
                w[ii % 16, ii // 16] = seg
                cols_list.append(np.tile(w, (8, 1)))
        idx_in = np.concatenate(cols_list, axis=1) if cols_list else np.zeros((P, 1), np.int16)

        # per-core node data (permuted into slot order)
        nbase = c * npc
        degc = np.ones((npc,), np.float32)
        hi = min(nbase + npc, N)
        if hi > nbase:
            degc[:hi - nbase] = deg_full[nbase:hi]
        bl = np.full((npc,), SENT, np.float32)
        if hi > nbase:
            bl[:hi - nbase] = batch[nbase:hi].astype(np.float32)
        degc = degc.reshape(bpc, P)[perm[c]].reshape(npc)
        bl = bl.reshape(bpc, P)[perm[c]].reshape(npc)
        deg_t = degc.reshape(bpc, P).T.copy()
        bl_t = bl.reshape(bpc, P).T.copy()

        per_core.append(dict(idx=idx_in, dstloc=dstloc, deg=deg_t, batchloc=bl_t))

    # fixed dst-lane pattern for the K regular chunks
    s_ar = np.arange(P)
    fixed_dl = np.empty((P, KSLOT), np.float32)
    for r in range(KSLOT):
        fixed_dl[:, r] = r * LPC + s_ar // KSLOT

    cnts = np.bincount(batch, minlength=NG).astype(np.float32)

    xT = np.zeros((IN_CH, Np), BF)
    xT[:, :N] = np.asarray(x, np.float32).T.astype(BF)
    colperm = np.empty((Np,), np.int64)
    for c in range(NCORES):
        base = c * npc
        colperm[base:base + npc] = base + (perm[c][:, None] * P +
                                           np.arange(P)[None, :]).reshape(-1)
    xT = xT[:, colperm]
    return plan, per_core, fixed_dl, cnts, xT


def build(plan: Plan, with_b1, with_b2):
    import os
    tab_hwdge = int(os.environ.get("GNN_TAB_HWDGE", "1"))
    stagec_act = int(os.environ.get("GNN_STAGEC_ACT", "1"))
    _install_patched_gather()
    nc = bacc.Bacc("TRN2", target_bir_lowering=False, debug=False,
                   num_swdge_queues=1, dynamic_dma_scratch_size=65536)
    f32, bf16, i16 = mybir.dt.float32, mybir.dt.bfloat16, mybir.dt.int16
    fp8 = mybir.dt.float8e4
    A = mybir.AluOpType
    AF = mybir.ActivationFunctionType
    npc, bpc = plan.npc, plan.bpc
    Np, gsize, win = plan.Np, plan.gsize, plan.win
    EL = int(os.environ.get("GNN_EL", "32"))  # gather payload bytes (fp8 elems)
    TABW = 256  # fp8 elems per table row (256B stride)
    NSP = max(plan.nsp_cols, 1)

    xT = nc.dram_tensor("xT", [IN_CH, npc], bf16, kind="ExternalInput")
    W1 = nc.dram_tensor("W1", [IN_CH, HID], f32, kind="ExternalInput")
    W2 = nc.dram_tensor("W2", [HID, OUT], f32, kind="ExternalInput")
    b1 = nc.dram_tensor("b1", [1, HID], f32, kind="ExternalInput")
    b2 = nc.dram_tensor("b2", [1, OUT], f32, kind="ExternalInput")
    deg = nc.dram_tensor("deg", [P, bpc], f32, kind="ExternalInput")
    dstloc = nc.dram_tensor("dstloc", [P, NSP], f32, kind="ExternalInput")
    fixeddl = nc.dram_tensor("fixeddl", [P, KSLOT], f32, kind="ExternalInput")
    batchloc = nc.dram_tensor("batchloc", [P, bpc], f32, kind="ExternalInput")
    idx = nc.dram_tensor("idx", [P, plan.idx_cols], i16, kind="ExternalInput")
    out = nc.dram_tensor("out", [NG, OUT], f32, kind="ExternalOutput")

    with tile.TileContext(nc) as tc:
        with tc.tile_pool(name="const", bufs=1) as cpool, \
             tc.tile_pool(name="persist", bufs=1) as pers, \
             tc.tile_pool(name="gth", bufs=2) as gpool, \
             tc.tile_pool(name="spool", bufs=16) as spool, \
             tc.tile_pool(name="psA", bufs=2, space="PSUM") as psA, \
             tc.tile_pool(name="psB", bufs=4, space="PSUM") as psB, \
             tc.tile_pool(name="psE", bufs=1, space="PSUM") as psE, \
             tc.tile_pool(name="dram", bufs=1, space="DRAM") as dram:

            # ---- constants (loads on scalar queue; xT slabs go on sync) ----
            iota_i = cpool.tile([P, P], mybir.dt.int32)
            nc.gpsimd.iota(iota_i[:], pattern=[[1, P]], base=0, channel_multiplier=0)
            iota_f = cpool.tile([P, P], f32)
            nc.vector.tensor_copy(out=iota_f[:], in_=iota_i[:])
            iota_bf = cpool.tile([P, P], bf16)
            nc.vector.tensor_copy(out=iota_bf[:], in_=iota_f[:])
            giota_bf = cpool.tile([P, NG], bf16)
            nc.vector.tensor_copy(out=giota_bf[:], in_=iota_f[:, :NG])
            ident_f = cpool.tile([P, P], f32)
            nc.gpsimd.memset(ident_f[:], 0.0)
            nc.gpsimd.affine_select(
                out=ident_f[:], in_=ident_f[:], compare_op=A.not_equal,
                fill=1.0, base=0, pattern=[[-1, P]], channel_multiplier=1)
            ident_bf = cpool.tile([P, P], bf16)
            nc.vector.tensor_copy(out=ident_bf[:], in_=ident_f[:])

            w1f = cpool.tile([P, 2 * HID], f32)
            nc.scalar.dma_start(out=w1f[:, 0:HID], in_=W1[0:P, :])
            nc.scalar.dma_start(out=w1f[:, HID:2 * HID], in_=W1[P:2 * P, :])
            w1t = cpool.tile([P, 2 * HID], bf16)
            nc.vector.tensor_copy(out=w1t[:], in_=w1f[:])
            w2f = cpool.tile([HID, OUT], f32)
            nc.scalar.dma_start(out=w2f[:], in_=W2[:, :])
            w2t = cpool.tile([HID, OUT], bf16)
            nc.vector.tensor_copy(out=w2t[:], in_=w2f[:])
            b1t = cpool.tile([1, HID], f32)
            nc.scalar.dma_start(out=b1t[:], in_=b1[:, :])
            b2t = cpool.tile([1, OUT], f32)
            nc.scalar.dma_start(out=b2t[:], in_=b2[:, :])

            deg_t = cpool.tile([P, bpc], f32)
            nc.scalar.dma_start(out=deg_t[:], in_=deg[:, :])
            rdeg = cpool.tile([P, bpc], f32)
            nc.vector.reciprocal(out=rdeg[:], in_=deg_t[:])
            dinv = cpool.tile([P, bpc], f32)
            nc.scalar.activation(out=dinv[:], in_=rdeg[:], func=AF.Sqrt)

            dl_t = pers.tile([P, NSP], f32)
            nc.scalar.dma_start(out=dl_t[:], in_=dstloc[:, :])
            fdl_t = cpool.tile([P, KSLOT], f32)
            nc.scalar.dma_start(out=fdl_t[:], in_=fixeddl[:, :])
            bl_t = cpool.tile([P, bpc], f32)
            nc.scalar.dma_start(out=bl_t[:], in_=batchloc[:, :])
            idx_t = pers.tile([P, plan.idx_cols], i16)
            nc.scalar.dma_start(out=idx_t[:], in_=idx[:, :])

            # fixed one-hot S for the K regular chunk shapes
            S_fixed = cpool.tile([P, KSLOT * P], bf16)
            for r in range(KSLOT):
                nc.vector.tensor_scalar(
                    out=S_fixed[:, r * P:(r + 1) * P], in0=iota_bf[:],
                    scalar1=fdl_t[:, r:r + 1], scalar2=None, op0=A.is_equal)

            # zero tile for table spare rows
            zero32 = cpool.tile([P, EL], fp8)
            nc.gpsimd.memset(zero32[:], 0.0)

            if with_b1:
                b1b_ps = psA.tile([P, HID], f32, tag="pst")
                ones_col = cpool.tile([1, P], f32)
                nc.gpsimd.memset(ones_col[:], 1.0)
                nc.tensor.matmul(out=b1b_ps[:], lhsT=ones_col[:], rhs=b1t[:],
                                 start=True, stop=True)
                b1b = cpool.tile([P, HID], f32)
                nc.vector.tensor_copy(out=b1b[:], in_=b1b_ps[:])
            if with_b2:
                b2b_ps = psA.tile([P, OUT], f32, tag="pst")
                ones_col2 = cpool.tile([1, P], f32)
                nc.gpsimd.memset(ones_col2[:], 1.0)
                nc.tensor.matmul(out=b2b_ps[:], lhsT=ones_col2[:], rhs=b2t[:],
                                 start=True, stop=True)
                b2b = cpool.tile([P, OUT], f32)
                nc.vector.tensor_copy(out=b2b[:], in_=b2b_ps[:])

            # ---- persistent activations ----
            y1_sh = pers.tile([P, bpc * HID], fp8)
            h1_sh = pers.tile([P, bpc * HID], bf16)
            y2_sh = pers.tile([P, bpc * OUT], fp8)
            h2_sh = pers.tile([P, bpc * OUT], bf16)

            # ---- DRAM internals ----
            TROWS = NGROUPS * win
            y1cc = dram.tile([P, bpc * HID], fp8, name="y1cc")
            y1full = dram.tile([NCORES * P, bpc * HID], fp8, addr_space="Shared",
                               name="y1full")
            y1tab = dram.tile([TROWS, TABW], fp8, name="y1tab")
            y2cc = dram.tile([P, bpc * OUT], fp8, name="y2cc")
            y2full = dram.tile([NCORES * P, bpc * OUT], fp8, addr_space="Shared",
                               name="y2full")
            y2tab = dram.tile([TROWS, TABW], fp8, name="y2tab")

            # zero spare rows of both tables (once)
            for g in range(NGROUPS):
                base = g * win + gsize
                nc.scalar.dma_start(out=y1tab[base:base + ZROWS, 0:EL], in_=zero32[:])
                nc.scalar.dma_start(out=y2tab[base:base + ZROWS, 0:EL], in_=zero32[:])

            # ---- stage A: y1 = (x @ W1) * dinv ----
            SLAB = 8
            sbA_ctx = tc.tile_pool(name="sbA", bufs=2)
            sbA = sbA_ctx.__enter__()
            for s0 in range(0, bpc, SLAB):
                sb = min(SLAB, bpc - s0)
                xt0 = sbA.tile([P, SLAB * P], bf16, tag="xt0")
                xt1 = sbA.tile([P, SLAB * P], bf16, tag="xt1")
                nc.sync.dma_start(out=xt0[:, :sb * P], in_=xT[0:P, s0 * P:(s0 + sb) * P])
                nc.sync.dma_start(out=xt1[:, :sb * P], in_=xT[P:2 * P, s0 * P:(s0 + sb) * P])
                for j in range(sb):
                    b = s0 + j
                    ps = psA.tile([P, HID], f32, name="psA_t", tag="pst")
                    nc.tensor.matmul(out=ps[:], lhsT=xt0[:, j * P:(j + 1) * P],
                                     rhs=w1t[:, 0:HID], start=True, stop=False)
                    nc.tensor.matmul(out=ps[:], lhsT=xt1[:, j * P:(j + 1) * P],
                                     rhs=w1t[:, HID:2 * HID], start=False, stop=True)
                    nc.vector.tensor_scalar(
                        out=y1_sh[:, b * HID:(b + 1) * HID], in0=ps[:],
                        scalar1=dinv[:, b:b + 1], scalar2=None, op0=A.mult)
            sbA_ctx.__exit__(None, None, None)

            def distribute(ysh, ycc, yfull, ytab, C):
                nc.sync.dma_start(out=ycc[:], in_=ysh[:])
                nc.gpsimd.collective_compute(
                    "AllGather", A.bypass, ins=[ycc[:]], outs=[yfull[:]],
                    replica_groups=[list(range(NCORES))])
                for cexp in range(NCORES):
                    g = cexp // 2
                    base = g * win + (cexp % 2) * npc
                    eng = nc.sync if cexp % 2 == 0 else nc.scalar
                    eng.dma_start(
                        out=ytab[base:base + npc, 0:C].rearrange(
                            "(b p) ch -> b p ch", p=P),
                        in_=yfull[cexp * P:(cexp + 1) * P, :].rearrange(
                            "p (b ch) -> b p ch", ch=C))

            distribute(y1_sh, y1cc, y1full, y1tab, HID)

            # ---- aggregation layers ----
            def agg_layer(ytab, ysh, hsh, C, with_b, bb, post_block):
                gstate = [dict(call=-1, tile=None) for _ in range(NGROUPS)]

                def ensure_call(g, q):
                    cidx = 0
                    for i, (q0, nch) in enumerate(plan.call_plan[g]):
                        if q0 <= q < q0 + nch:
                            cidx = i
                            break
                    st = gstate[g]
                    if st["call"] == cidx:
                        return st["tile"], plan.call_plan[g][cidx][0]
                    (q0, nch) = plan.call_plan[g][cidx]
                    gt = gpool.tile([P, plan.maxch * EL], fp8, tag=f"g{g}",
                                    name=f"gt{g}")
                    col = plan.call_col[g][cidx]
                    nid = nch * P
                    nc.gpsimd.dma_gather_p(
                        out_ap=gt[:, :nch * EL].rearrange("p (k c) -> p k c", c=EL),
                        in_ap=ytab[g * win: g * win + gsize + ZROWS, 0:EL],
                        idxs_ap=idx_t[:, col:col + nid // 16],
                        num_idxs=nid, num_idxs_reg=nid,
                        elem_size=EL, elem_step=TABW, single_packet=False)
                    st["call"] = cidx
                    st["tile"] = gt
                    return gt, q0

                qcol = 0  # spill dstloc column counter (b-major, g, j)
                for b in range(bpc):
                    ps = psB.tile([P, C], f32, name="psB_t")
                    nchunks = sum(plan.nch_bg[b])
                    nc.tensor.matmul(out=ps[:], lhsT=ident_bf[:],
                                     rhs=ysh[:, b * C:(b + 1) * C],
                                     start=True, stop=False)
                    done = 0
                    for g in range(NGROUPS):
                        for j in range(plan.nch_bg[b][g]):
                            q = int(plan.prefix_g[b, g]) + j
                            gt, q0 = ensure_call(g, q)
                            sl = q - q0
                            if j < KSLOT:
                                S_ap = S_fixed[:, j * P:(j + 1) * P]
                            else:
                                S = spool.tile([P, P], bf16, name="S_t")
                                nc.vector.tensor_scalar(
                                    out=S[:], in0=iota_bf[:],
                                    scalar1=dl_t[:, qcol:qcol + 1], scalar2=None,
                                    op0=A.is_equal)
                                qcol += 1
                                S_ap = S[:]
                            done += 1
                            nc.tensor.matmul(
                                out=ps[:], lhsT=S_ap,
                                rhs=gt[:, sl * EL: sl * EL + C],
                                start=False, stop=(done == nchunks))
                    if with_b:
                        t1 = spool.tile([P, C], f32, name="t1_t")
                        nc.vector.tensor_scalar(
                            out=t1[:], in0=ps[:], scalar1=dinv[:, b:b + 1],
                            scalar2=None, op0=A.mult)
                        t2 = spool.tile([P, C], f32, name="t2_t")
                        nc.vector.tensor_tensor(out=t2[:], in0=t1[:],
                                                in1=bb[:, :C], op=A.add)
                        nc.scalar.activation(out=hsh[:, b * C:(b + 1) * C],
                                             in_=t2[:], func=AF.Relu)
                    else:
                        nc.vector.tensor_scalar(
                            out=hsh[:, b * C:(b + 1) * C], in0=ps[:],
                            scalar1=dinv[:, b:b + 1], scalar2=0.0,
                            op0=A.mult, op1=A.max)
                    if post_block is not None:
                        post_block(b)

            # stage C per block: transpose h1 block, xw2, scale -> y2_sh
            def stage_c(b):
                tp = psA.tile([HID, P], bf16, name="psT_t", tag="pst")
                nc.tensor.transpose(out=tp[:], in_=h1_sh[:, b * HID:(b + 1) * HID],
                                    identity=ident_bf[:])
                h1Tb = spool.tile([HID, P], bf16, name="h1Tb_t")
                nc.scalar.activation(out=h1Tb[:], in_=tp[:], func=AF.Copy)
                ps2 = psA.tile([P, OUT], f32, name="ps2_t", tag="pst")
                nc.tensor.matmul(out=ps2[:], lhsT=h1Tb[:],
                                 rhs=w2t[:], start=True, stop=True)
                nc.vector.tensor_scalar(
                    out=y2_sh[:, b * OUT:(b + 1) * OUT], in0=ps2[:],
                    scalar1=dinv[:, b:b + 1], scalar2=None, op0=A.mult)

            agg_layer(y1tab, y1_sh, h1_sh, HID, with_b1,
                      b1b if with_b1 else None, stage_c)

            distribute(y2_sh, y2cc, y2full, y2tab, OUT)

            # layer 2 + pooling per block
            pp = psE.tile([NG, OUT], f32)

            def pool_block(b):
                Sb = spool.tile([P, NG], bf16, name="Sb_t")
                nc.vector.tensor_scalar(
                    out=Sb[:], in0=giota_bf[:], scalar1=bl_t[:, b:b + 1],
                    scalar2=None, op0=A.is_equal)
                nc.tensor.matmul(out=pp[:], lhsT=Sb[:],
                                 rhs=h2_sh[:, b * OUT:(b + 1) * OUT],
                                 start=(b == 0), stop=(b == bpc - 1))

            agg_layer(y2tab, y2_sh, h2_sh, OUT, with_b2,
                      b2b if with_b2 else None, pool_block)

            # ---- partial pooled sums out (host finishes mean + log_softmax) ----
            sums = cpool.tile([NG, OUT], f32)
            nc.vector.tensor_copy(out=sums[:], in_=pp[:])
            nc.sync.dma_start(out=out[:, :], in_=sums[:])

    nc.compile()
    return nc


def run_gnn(x, edge_index, batch, W1, b1, W2, b2, runner_cls=None, time_iters=0):
    from concourse.bass_utils import run_bass_kernel_spmd
    plan, per_core, fixed_dl, cnts, xT = host_prep(x, edge_index, batch)
    with_b1 = bool(np.any(np.asarray(b1)))
    with_b2 = bool(np.any(np.asarray(b2)))
    nc = build(plan, with_b1, with_b2)
    in_maps = []
    for c in range(NCORES):
        m = dict(per_core[c])
        m["xT"] = np.ascontiguousarray(xT[:, c * plan.npc:(c + 1) * plan.npc])
        m["W1"] = np.asarray(W1, np.float32)
        m["W2"] = np.asarray(W2, np.float32)
        m["b1"] = np.asarray(b1, np.float32).reshape(1, HID)
        m["b2"] = np.asarray(b2, np.float32).reshape(1, OUT)
        m["fixeddl"] = fixed_dl
        in_maps.append(m)
    res = run_bass_kernel_spmd(nc, in_maps, core_ids=list(range(NCORES)))
    total = np.zeros((NG, OUT), np.float64)
    for c in range(NCORES):
        total += np.asarray(res.results[c]["out"], np.float64)
    pooled = total / np.maximum(cnts, 1.0)[:, None]
    z = pooled - pooled.max(axis=1, keepdims=True)
    ls = z - np.log(np.exp(z).sum(axis=1, keepdims=True))
    return ls.astype(np.float32)


def kernel(x, edge_index, batch, W1, b1, W2, b2):
    """Full-input 2-layer GCN + mean-pool + log_softmax on 8 trn2 NeuronCores."""
    return np.asarray(
        run_gnn(np.asarray(x), np.asarray(edge_index), np.asarray(batch),
                np.asarray(W1), np.asarray(b1), np.asarray(W2), np.asarray(b2)),
        dtype=np.float32)
